# revision 16
# baseline (speedup 1.0000x reference)
"""Trainium2 Bass kernel for per-node rank-1 self-attention (NodeFeatureSelfAttention).

Math: for each node n (row of x):
    q = s*(Wq @ xp + bq); k = Wk @ xp + bk; v = Wv @ xp + bv   (xp = x + pe)
    out[i] = sum_j softmax_j(q_i * k_j)[j] * v_j = g(q_i)
with g(t) = sum_j exp(t*k_j)*v_j / sum_j exp(t*k_j), a smooth scalar function
per node. We sample g at M shared Chebyshev points t_m (ACT engine exps),
reduce with masked-ones matmuls on the PE, turn samples directly into
broadcast monomial-coefficient planes ckf[node, tile, k, i] with one small
PE matmul per tile (Ainv folded into a constant selection matrix), and
evaluate the degree-(M-1) interpolant with full-width tensor_tensor Horner
steps column-split across DVE (bf16) and GPSIMD.

Data-parallel over nodes across 8 NeuronCores; weights replicated.
"""
import sys
sys.path.insert(0, "/opt/trn_rl_repo")
import numpy as np
from contextlib import ExitStack

N, D = 16384, 128
NCORES = 8
NLOC = N // NCORES            # 2048 nodes per core
NT = NLOC // 128              # 16 node-tiles per core
M = 4                         # Chebyshev sample count (degree M-1 interpolant)
NT_GP = 8                     # tiles on DVE chain A (GPSIMD contends with DVE for
NT_DA = 8                     # SBUF ports, so phase D runs on DVE only, 2 chains)

_built = {}


def _build():
    """Build + finalize the (data-independent) bass module once."""
    if "nc" in _built:
        return _built["nc"]
    import concourse.bacc as bacc
    import concourse.tile as tile
    from concourse import mybir

    f32 = mybir.dt.float32
    bf16 = mybir.dt.bfloat16
    nc = bacc.Bacc()

    xs = nc.declare_dram_parameter("xs", [NLOC, D], f32, isOutput=False)
    # f32 consts: TMS M | BIASCOL 2
    NC32 = M + 2
    CONSTS = nc.declare_dram_parameter("CONSTS", [D, NC32], f32, isOutput=False)
    # bf16 consts: IDN D | WQT D | WKT D | WVT D | FMASK 8D | MASKS 32M | BQB D | RSEL M*128
    NCB = 4 * D + 8 * D + 32 * M + D + M * 128
    CONSTB = nc.declare_dram_parameter("CONSTB", [D, NCB], bf16, isOutput=False)
    OUT = nc.declare_dram_parameter("out", [NLOC, D], f32, isOutput=True)

    with tile.TileContext(nc) as tc, ExitStack() as ctx:
        from concourse.mybir import AluOpType
        singles = ctx.enter_context(tc.tile_pool(name="singles", bufs=1))
        emp = ctx.enter_context(tc.tile_pool(name="emp", bufs=4))
        evp = ctx.enter_context(tc.tile_pool(name="evp", bufs=4))

        # ---- DMA in: x chunk 0 + small consts first, rest interleaved ----
        x_sb = singles.tile([D, NT, D], f32)
        xs_r = xs.rearrange("(t p) d -> p t d", p=128)
        cblob = singles.tile([D, NC32], f32)
        bblob = singles.tile([D, NCB], bf16)
        nc.sync.dma_start(out=x_sb[:, 0:2, :], in_=xs_r[:, 0:2, :])
        nc.scalar.dma_start(out=cblob[:, :], in_=CONSTS[:, :])
        nc.gpsimd.dma_start(out=x_sb[:, 4:6, :], in_=xs_r[:, 4:6, :])
        nc.scalar.dma_start(out=bblob[:, 0:4 * D], in_=CONSTB[:, 0:4 * D])
        nc.sync.dma_start(out=x_sb[:, 2:4, :], in_=xs_r[:, 2:4, :])
        nc.gpsimd.dma_start(out=x_sb[:, 10:12, :], in_=xs_r[:, 10:12, :])
        nc.scalar.dma_start(out=x_sb[:, 6:8, :], in_=xs_r[:, 6:8, :])
        nc.sync.dma_start(out=x_sb[:, 8:10, :], in_=xs_r[:, 8:10, :])
        nc.gpsimd.dma_start(out=x_sb[:, 14:16, :], in_=xs_r[:, 14:16, :])
        nc.scalar.dma_start(out=bblob[:, 4 * D:NCB], in_=CONSTB[:, 4 * D:NCB])
        nc.sync.dma_start(out=x_sb[:, 12:14, :], in_=xs_r[:, 12:14, :])

        o = 0
        tms = cblob[:, o:o + M]; o += M
        biascol = cblob[:, o:o + 2]; o += 2
        ob = 0
        idn = bblob[:, ob:ob + D]; ob += D
        wqt = bblob[:, ob:ob + D]; ob += D
        wkt = bblob[:, ob:ob + D]; ob += D
        wvt = bblob[:, ob:ob + D]; ob += D
        fmask = bblob[:, ob:ob + 8 * D].rearrange("p (i c) -> p i c", i=8); ob += 8 * D
        masks = bblob[:, ob:ob + 32 * M].rearrange("p (i c) -> p i c", i=M); ob += 32 * M
        bqb = bblob[:, ob:ob + D]; ob += D
        rsel = bblob[:, ob:ob + M * 128]   # [32grp+m, (k,i)] = Ainv[k,m], per grp
        ob += M * 128

        x_bf = singles.tile([D, NT, D], bf16)         # x cast to bf16 (GPSIMD)
        xT_all = singles.tile([D, NT, 128], bf16)     # x^T per tile (bf16)
        q_all = singles.tile([D, NT, 128], bf16)      # Q' [node_p, t, i]
        kvt = singles.tile([D, NLOC], bf16)           # K^T [j, n]
        vt_b = singles.tile([D, NLOC], bf16)          # V^T
        rden = singles.tile([D, NLOC], f32)
        g_sb = singles.tile([D, NLOC], bf16)
        ckf = singles.tile([D, NT, M, 128], bf16)     # coeff planes [node_p, t, k, i]
        out_sb = singles.tile([D, NT, 128], f32)

        psA_cm = tc.tile_pool(name="psA", bufs=1, space="PSUM")
        psA = psA_cm.__enter__()

        # ---- Phase A1: bf16 casts (DVE), transposes + K/V ----
        for c in range(8):
            nc.vector.tensor_copy(x_bf[:, 2 * c:2 * c + 2, :], x_sb[:, 2 * c:2 * c + 2, :])

        def a1_quad(qd):
            for t in range(4 * qd, 4 * qd + 4):
                xt_ps = psA.tile([D, 128], bf16, tag=f"xtps{t % 2}", name=f"xtps{t}")
                nc.tensor.transpose(xt_ps, x_bf[:, t, :], idn)
                nc.vector.tensor_copy(xT_all[:, t, :], xt_ps)
            xT4 = xT_all[:, 4 * qd:4 * qd + 4, :]
            nsl = slice(qd * 512, (qd + 1) * 512)
            k_ps = psA.tile([128, 512], f32, tag="kps", name=f"kps{qd}", bufs=2)
            v_ps = psA.tile([128, 512], f32, tag="vps", name=f"vps{qd}", bufs=2)
            nc.tensor.matmul(k_ps, wkt, xT4, start=True, stop=True)
            nc.tensor.matmul(v_ps, wvt, xT4, start=True, stop=True)
            nc.scalar.add(out=kvt[:, nsl], in_=k_ps, add=biascol[:, 0:1])
            nc.scalar.add(out=vt_b[:, nsl], in_=v_ps, add=biascol[:, 1:2])

        ems = {}
        a1_quad(0)
        a1_quad(1)
        for m in range(M):
            em = emp.tile([D, 1024], bf16, name=f"em{m}h0")
            nc.scalar.activation(out=em, in_=kvt[:, 0:1024],
                                 func=mybir.ActivationFunctionType.Exp,
                                 scale=tms[:, m:m + 1])
            ems[(m, 0)] = em
        a1_quad(2)
        a1_quad(3)

        # ---- Phase A2: Q' tiles (needed only in phase D); bias via DVE add ----
        for t in range(NT):
            q_ps = psA.tile([128, D], f32, tag=f"qps{t % 2}", name=f"qps{t}")
            nc.tensor.matmul(q_ps, xT_all[:, t, :], wqt, start=True, stop=True)
            nc.vector.tensor_tensor(out=q_all[:, t, :], in0=q_ps, in1=bqb,
                                    op=AluOpType.add)
        psA_cm.__exit__(None, None, None)

        # ---- Phase B: half-split m-major exps + masked reduction matmuls ----
        psB_cm = tc.tile_pool(name="psB", bufs=1, space="PSUM")
        psB = psB_cm.__enter__()
        coef_ps = psB.tile([D, 2, NLOC], f32)
        for h in range(2):
            hsl = slice(h * 1024, (h + 1) * 1024)
            for m in range(M):
                if (m, h) in ems:
                    em = ems[(m, h)]
                else:
                    em = emp.tile([D, 1024], bf16, name=f"em{m}h{h}")
                    nc.scalar.activation(out=em, in_=kvt[:, hsl],
                                         func=mybir.ActivationFunctionType.Exp,
                                         scale=tms[:, m:m + 1])
                ev = evp.tile([D, 1024], bf16, name=f"ev{m}h{h}")
                nc.vector.tensor_mul(ev, em, vt_b[:, hsl])
                for jj in range(2):
                    j = 2 * h + jj
                    sl = slice(j * 512, (j + 1) * 512)
                    lsl = slice(jj * 512, (jj + 1) * 512)
                    if m == 0:
                        nc.tensor.matmul(coef_ps[:, 0, sl], fmask[:, j, :], ev[:, lsl],
                                         start=True, stop=False)
                    else:
                        nc.tensor.matmul(coef_ps[32 * j:32 * j + 32, 0, sl],
                                         masks[:, m, :], ev[:, lsl],
                                         start=False, stop=(m == M - 1),
                                         tile_position=(0, 32 * j))
                for jj in range(2):
                    j = 2 * h + jj
                    sl = slice(j * 512, (j + 1) * 512)
                    lsl = slice(jj * 512, (jj + 1) * 512)
                    if m == 0:
                        nc.tensor.matmul(coef_ps[:, 1, sl], fmask[:, 4 + j, :], em[:, lsl],
                                         start=True, stop=False)
                    else:
                        nc.tensor.matmul(coef_ps[32 * j:32 * j + 32, 1, sl],
                                         masks[:, m, :], em[:, lsl],
                                         start=False, stop=(m == M - 1),
                                         tile_position=(0, 32 * j))

        # ---- Phase C: g = num/den from PSUM, then per-tile ckf planes ----
        for j in range(4):
            nsl = slice(j * 512, (j + 1) * 512)
            nc.vector.reciprocal_approx_fast(out=rden[:, nsl], in_=coef_ps[:, 1, nsl])
            nc.vector.tensor_mul(g_sb[:, nsl], coef_ps[:, 0, nsl], rden[:, nsl])
        psB_cm.__exit__(None, None, None)
        psD = ctx.enter_context(tc.tile_pool(name="psD", bufs=1, space="PSUM"))
        for t in range(NT):
            j = t // 4
            ckf_ps = psD.tile([128, M * 128], f32, tag=f"ckfps{t % 4}", name=f"ckfps{t}")
            kw = {}
            if j == 3:
                kw["tile_position"] = (96, 0)
            nc.tensor.matmul(ckf_ps, g_sb[32 * j:32 * j + 32, t * 128:(t + 1) * 128],
                             rsel[32 * j:32 * j + 32, :], start=True, stop=True, **kw)
            if t % 2 == 0:
                nc.scalar.copy(out=ckf[:, t, :, :], in_=ckf_ps)
            else:
                nc.vector.tensor_copy(ckf[:, t, :, :], ckf_ps)

        # ---- Phase D: full-width Horner, column-split GPSIMD / DVE x2 ----
        hor = ctx.enter_context(tc.tile_pool(name="hor", bufs=1))
        spans = [(nc.vector, 0, 8),
                 (nc.vector, 8, 16)]
        chains = []
        for ci, (eng, ta, tb) in enumerate(spans):
            nt = tb - ta
            f0 = hor.tile([D, nt, 128], bf16, name=f"f{ci}0")
            f1 = hor.tile([D, nt, 128], bf16, name=f"f{ci}1")
            chains.append((eng, ta, tb, [f0, f1]))

        out_r = OUT.rearrange("(t p) d -> p t d", p=128)
        dma_engs = {0: nc.sync, 4: nc.scalar, 8: nc.sync, 12: nc.scalar}

        def step(ci, k, last):
            eng, ta, tb, fb = chains[ci]
            q = q_all[:, ta:tb, :]
            if k == M - 1:
                eng.tensor_tensor(out=fb[0], in0=q, in1=ckf[:, ta:tb, k, :],
                                  op=AluOpType.mult)
                return
            if not last:
                eng.tensor_tensor(out=fb[1], in0=fb[0], in1=ckf[:, ta:tb, k, :],
                                  op=AluOpType.add)
                eng.tensor_tensor(out=fb[0], in0=fb[1], in1=q, op=AluOpType.mult)
                return
            eng.tensor_tensor(out=out_sb[:, ta:tb, :], in0=fb[0],
                              in1=ckf[:, ta:tb, k, :], op=AluOpType.add)
            for ca in range(ta, tb, 4):
                dma_engs[ca].dma_start(out=out_r[:, ca:ca + 4, :],
                                       in_=out_sb[:, ca:ca + 4, :])

        for k in range(M - 1, -1, -1):
            for ci in range(len(spans)):
                step(ci, k, last=(k == 0))

    nc.finalize()
    _built["nc"] = nc
    return nc


def _host_prep(x, Wq, bq, Wk, bk, Wv, bv):
    """Fold positional encoding + scale into weights; build constants."""
    x = np.ascontiguousarray(x, dtype=np.float32)
    Wq = np.asarray(Wq, np.float32); bq = np.asarray(bq, np.float32)
    Wk = np.asarray(Wk, np.float32); bk = np.asarray(bk, np.float32)
    Wv = np.asarray(Wv, np.float32); bv = np.asarray(bv, np.float32)

    half = D // 2
    div = np.exp(np.arange(half, dtype=np.float64) * (-np.log(10000.0) / D))
    pe = np.zeros(D, np.float64)
    pe[0::2] = np.sin(np.arange(0, D, 2, dtype=np.float64) * div)
    pe[1::2] = np.cos(np.arange(1, D, 2, dtype=np.float64) * div)
    pe = pe.astype(np.float32)

    s = np.float32(1.0 / np.sqrt(D))
    Wq_s = (Wq * s).astype(np.float32)
    bq_s = (s * (bq + Wq @ pe)).astype(np.float32)
    bk_s = (bk + Wk @ pe).astype(np.float32)
    bv_s = (bv + Wv @ pe).astype(np.float32)

    # q' range for the Chebyshev interval
    Qp = x @ Wq_s.T + bq_s
    Tmax = float(np.abs(Qp).max()) * 1.0005

    theta = (2 * np.arange(M) + 1) * np.pi / (2 * M)
    tm = np.cos(theta) * Tmax                        # f64 Chebyshev points
    Vand = tm[:, None] ** np.arange(M)[None, :]
    Ainv = np.linalg.inv(Vand)                       # coeffs = Ainv @ g_samples

    masks = np.zeros((D, M, 32), np.float32)
    for mm in range(M):
        masks[:, mm, mm] = 1.0            # stream m -> in-group partition m
    fmask = np.zeros((8, D, D), np.float32)
    for j in range(4):
        fmask[j, :, 32 * j] = 1.0         # num m=0 -> partition 32j; other rows 0
        fmask[4 + j, :, :] = 1.0          # den m=0 -> every row gets a positive sum
        fmask[4 + j, :, 32 * j + 1:32 * j + M] = 0.0   # rows for m>=1 accumulate cleanly
    tms = np.tile(tm.astype(np.float32)[None, :], (D, 1))

    # rsel[32j+m, (k,i)] = Ainv[k, m]  (zero for m >= M): ckf = g_rows.T @ rsel
    rsel = np.zeros((D, M, 128), np.float32)
    for j in range(4):
        for m in range(M):
            for k in range(M):
                rsel[32 * j + m, k, :] = Ainv[k, m]

    blob32 = np.concatenate([
        tms,                                                        # TMS
        np.stack([bk_s, bv_s], axis=1),                             # BIASCOL
    ], axis=1).astype(np.float32)

    import ml_dtypes
    blobb = np.concatenate([
        np.eye(D, dtype=np.float32),                                # IDN
        np.ascontiguousarray(Wq_s.T),                               # WQT
        np.ascontiguousarray(Wk.T),                                 # WKT
        np.ascontiguousarray(Wv.T),                                 # WVT
        fmask.transpose(1, 0, 2).reshape(D, 8 * D),                 # FMASK
        masks.reshape(D, M * 32),                                   # MASKS
        np.tile(bq_s[None, :], (D, 1)),                             # BQB
        rsel.reshape(D, M * 128),                                   # RSEL
    ], axis=1).astype(ml_dtypes.bfloat16)

    consts = {"CONSTS": np.ascontiguousarray(blob32),
              "CONSTB": np.ascontiguousarray(blobb)}
    return x, consts


def _run(inputs, trace=False):
    from concourse.bass_utils import run_bass_kernel_spmd
    x, consts = _host_prep(**inputs)
    nc = _build()
    in_maps = []
    for i in range(NCORES):
        m = {"xs": np.ascontiguousarray(x[i * NLOC:(i + 1) * NLOC])}
        m.update(consts)
        in_maps.append(m)
    res = run_bass_kernel_spmd(nc, in_maps, list(range(NCORES)), trace=trace)
    out = np.concatenate([r["out"] for r in res.results], axis=0)
    return out, res.exec_time_ns


def kernel(**inputs):
    out, _ = _run(inputs, trace=False)
    return out


# revision 18
# speedup vs baseline: 1.0638x; 1.0638x over previous
"""Trainium2 Bass kernel for per-node rank-1 self-attention (NodeFeatureSelfAttention).

Math: for each node n (row of x):
    q = s*(Wq @ xp + bq); k = Wk @ xp + bk; v = Wv @ xp + bv   (xp = x + pe)
    out[i] = sum_j softmax_j(q_i * k_j)[j] * v_j = g(q_i)
with g(t) = sum_j exp(t*k_j)*v_j / sum_j exp(t*k_j), a smooth scalar function
per node. We sample g at M shared Chebyshev points t_m (ACT engine exps),
reduce with masked-ones matmuls on the PE, turn samples directly into
broadcast monomial-coefficient planes ckf[node, tile, k, i] with one small
PE matmul per tile (Ainv folded into a constant selection matrix), and
evaluate the degree-(M-1) interpolant with full-width tensor_tensor Horner
steps column-split across DVE (bf16) and GPSIMD.

Data-parallel over nodes across 8 NeuronCores; weights replicated.
"""
import sys
sys.path.insert(0, "/opt/trn_rl_repo")
import numpy as np
from contextlib import ExitStack

N, D = 16384, 128
NCORES = 8
NLOC = N // NCORES            # 2048 nodes per core
NT = NLOC // 128              # 16 node-tiles per core
M = 4                         # Chebyshev sample count (degree M-1 interpolant)
NT_GP = 4                     # leading tiles evaluated on GPSIMD
NT_DA = 6                     # next tiles on DVE chain A (rest on chain B)

_built = {}


def _build():
    """Build + finalize the (data-independent) bass module once."""
    if "nc" in _built:
        return _built["nc"]
    import concourse.bacc as bacc
    import concourse.tile as tile
    from concourse import mybir

    f32 = mybir.dt.float32
    bf16 = mybir.dt.bfloat16
    nc = bacc.Bacc()

    xs = nc.declare_dram_parameter("xs", [NLOC, D], f32, isOutput=False)
    # f32 consts: IDN D | TMS M | BIASCOL 2
    NC32 = D + M + 2
    CONSTS = nc.declare_dram_parameter("CONSTS", [D, NC32], f32, isOutput=False)
    # bf16 consts: WQT D | WKT D | WVT D | FMASK 8D | MASKS 32M | BQB D | RSEL M*128
    NCB = 3 * D + 8 * D + 32 * M + D + M * 128
    CONSTB = nc.declare_dram_parameter("CONSTB", [D, NCB], bf16, isOutput=False)
    OUT = nc.declare_dram_parameter("out", [NLOC, D], f32, isOutput=True)

    with tile.TileContext(nc) as tc, ExitStack() as ctx:
        from concourse.mybir import AluOpType
        singles = ctx.enter_context(tc.tile_pool(name="singles", bufs=1))
        emp = ctx.enter_context(tc.tile_pool(name="emp", bufs=4))
        evp = ctx.enter_context(tc.tile_pool(name="evp", bufs=4))

        # ---- DMA in: x chunk 0 first (transposes), consts split across queues
        x_sb = singles.tile([D, NT, D], f32)
        xs_r = xs.rearrange("(t p) d -> p t d", p=128)
        cblob = singles.tile([D, NC32], f32)
        bblob = singles.tile([D, NCB], bf16)
        nc.sync.dma_start(out=x_sb[:, 0:4, :], in_=xs_r[:, 0:4, :])
        nc.scalar.dma_start(out=cblob[:, :], in_=CONSTS[:, :])
        nc.sync.dma_start(out=x_sb[:, 4:8, :], in_=xs_r[:, 4:8, :])
        nc.scalar.dma_start(out=bblob[:, 0:12 * D], in_=CONSTB[:, 0:12 * D])
        nc.sync.dma_start(out=x_sb[:, 8:12, :], in_=xs_r[:, 8:12, :])
        nc.scalar.dma_start(out=bblob[:, 12 * D:NCB], in_=CONSTB[:, 12 * D:NCB])
        nc.sync.dma_start(out=x_sb[:, 12:16, :], in_=xs_r[:, 12:16, :])

        o = 0
        idn = cblob[:, o:o + D]; o += D
        tms = cblob[:, o:o + M]; o += M
        biascol = cblob[:, o:o + 2]; o += 2
        ob = 0
        wqt = bblob[:, ob:ob + D]; ob += D
        wkt = bblob[:, ob:ob + D]; ob += D
        wvt = bblob[:, ob:ob + D]; ob += D
        fmask = bblob[:, ob:ob + 8 * D].rearrange("p (i c) -> p i c", i=8); ob += 8 * D
        masks = bblob[:, ob:ob + 32 * M].rearrange("p (i c) -> p i c", i=M); ob += 32 * M
        bqb = bblob[:, ob:ob + D]; ob += D
        rsel = bblob[:, ob:ob + M * 128]   # [32grp+m, (k,i)] = Ainv[k,m], per grp
        ob += M * 128

        xT_all = singles.tile([D, NT, 128], bf16)     # x^T per tile (bf16)
        q_all = singles.tile([D, NT, 128], bf16)      # Q' [node_p, t, i]
        kvt = singles.tile([D, NLOC], bf16)           # K^T [j, n]
        vt_b = singles.tile([D, NLOC], bf16)          # V^T
        rden = singles.tile([D, NLOC], f32)
        g_sb = singles.tile([D, NLOC], bf16)
        ckf = singles.tile([D, NT, M, 128], bf16)     # coeff planes [node_p, t, k, i]
        out_sb = singles.tile([D, NT, 128], f32)

        psA_cm = tc.tile_pool(name="psA", bufs=1, space="PSUM")
        psA = psA_cm.__enter__()

        # ---- Phase A1: transposes + K/V (kvt complete ASAP for phase B) ----
        for qd in range(4):
            for t in range(4 * qd, 4 * qd + 4):
                xt_ps = psA.tile([D, 128], f32, tag=f"xtps{t % 2}", name=f"xtps{t}")
                nc.tensor.transpose(xt_ps, x_sb[:, t, :], idn)
                nc.vector.tensor_copy(xT_all[:, t, :], xt_ps)
            xT4 = xT_all[:, 4 * qd:4 * qd + 4, :]
            nsl = slice(qd * 512, (qd + 1) * 512)
            k_ps = psA.tile([128, 512], f32, tag="kps", name=f"kps{qd}", bufs=2)
            v_ps = psA.tile([128, 512], f32, tag="vps", name=f"vps{qd}", bufs=2)
            nc.tensor.matmul(k_ps, wkt, xT4, start=True, stop=True)
            nc.tensor.matmul(v_ps, wvt, xT4, start=True, stop=True)
            nc.scalar.add(out=kvt[:, nsl], in_=k_ps, add=biascol[:, 0:1])
            nc.scalar.add(out=vt_b[:, nsl], in_=v_ps, add=biascol[:, 1:2])

        # ---- Phase A2: Q' tiles (needed only in phase D); bias via DVE add ----
        for t in range(NT):
            q_ps = psA.tile([128, D], f32, tag=f"qps{t % 2}", name=f"qps{t}")
            nc.tensor.matmul(q_ps, xT_all[:, t, :], wqt, start=True, stop=True)
            nc.vector.tensor_tensor(out=q_all[:, t, :], in0=q_ps, in1=bqb,
                                    op=AluOpType.add)
        psA_cm.__exit__(None, None, None)

        # ---- Phase B: half-split m-major exps + masked reduction matmuls ----
        psB_cm = tc.tile_pool(name="psB", bufs=1, space="PSUM")
        psB = psB_cm.__enter__()
        coef_ps = psB.tile([D, 2, NLOC], f32)
        for h in range(2):
            hsl = slice(h * 1024, (h + 1) * 1024)
            for m in range(M):
                em = emp.tile([D, 1024], bf16, name=f"em{m}h{h}")
                nc.scalar.activation(out=em, in_=kvt[:, hsl],
                                     func=mybir.ActivationFunctionType.Exp,
                                     scale=tms[:, m:m + 1])
                ev = evp.tile([D, 1024], bf16, name=f"ev{m}h{h}")
                nc.vector.tensor_mul(ev, em, vt_b[:, hsl])
                for jj in range(2):
                    j = 2 * h + jj
                    sl = slice(j * 512, (j + 1) * 512)
                    lsl = slice(jj * 512, (jj + 1) * 512)
                    if m == 0:
                        nc.tensor.matmul(coef_ps[:, 0, sl], fmask[:, j, :], ev[:, lsl],
                                         start=True, stop=False)
                    else:
                        nc.tensor.matmul(coef_ps[32 * j:32 * j + 32, 0, sl],
                                         masks[:, m, :], ev[:, lsl],
                                         start=False, stop=(m == M - 1),
                                         tile_position=(0, 32 * j))
                for jj in range(2):
                    j = 2 * h + jj
                    sl = slice(j * 512, (j + 1) * 512)
                    lsl = slice(jj * 512, (jj + 1) * 512)
                    if m == 0:
                        nc.tensor.matmul(coef_ps[:, 1, sl], fmask[:, 4 + j, :], em[:, lsl],
                                         start=True, stop=False)
                    else:
                        nc.tensor.matmul(coef_ps[32 * j:32 * j + 32, 1, sl],
                                         masks[:, m, :], em[:, lsl],
                                         start=False, stop=(m == M - 1),
                                         tile_position=(0, 32 * j))

        # ---- Phase C: g = num/den from PSUM, then per-tile ckf planes ----
        for j in range(4):
            nsl = slice(j * 512, (j + 1) * 512)
            nc.vector.reciprocal_approx_fast(out=rden[:, nsl], in_=coef_ps[:, 1, nsl])
            nc.vector.tensor_mul(g_sb[:, nsl], coef_ps[:, 0, nsl], rden[:, nsl])
        psB_cm.__exit__(None, None, None)
        psD = ctx.enter_context(tc.tile_pool(name="psD", bufs=1, space="PSUM"))
        for t in range(NT):
            j = t // 4
            ckf_ps = psD.tile([128, M * 128], f32, tag=f"ckfps{t % 4}", name=f"ckfps{t}")
            kw = {}
            if j == 3:
                kw["tile_position"] = (96, 0)
            nc.tensor.matmul(ckf_ps, g_sb[32 * j:32 * j + 32, t * 128:(t + 1) * 128],
                             rsel[32 * j:32 * j + 32, :], start=True, stop=True, **kw)
            if t % 2 == 0:
                nc.scalar.copy(out=ckf[:, t, :, :], in_=ckf_ps)
            else:
                nc.vector.tensor_copy(ckf[:, t, :, :], ckf_ps)

        # ---- Phase D: full-width Horner, column-split GPSIMD / DVE x2 ----
        hor = ctx.enter_context(tc.tile_pool(name="hor", bufs=1))
        spans = [(nc.gpsimd, 0, NT_GP),
                 (nc.vector, NT_GP, NT_GP + NT_DA),
                 (nc.vector, NT_GP + NT_DA, NT)]
        chains = []
        for ci, (eng, ta, tb) in enumerate(spans):
            nt = tb - ta
            f0 = hor.tile([D, nt, 128], bf16, name=f"f{ci}0")
            f1 = hor.tile([D, nt, 128], bf16, name=f"f{ci}1")
            chains.append((eng, ta, tb, [f0, f1]))

        def step(ci, k, last):
            eng, ta, tb, fb = chains[ci]
            q = q_all[:, ta:tb, :]
            if k == M - 1:
                eng.tensor_tensor(out=fb[0], in0=q, in1=ckf[:, ta:tb, k, :],
                                  op=AluOpType.mult)
                return
            dst = out_sb[:, ta:tb, :] if last else fb[1]
            eng.tensor_tensor(out=dst, in0=fb[0], in1=ckf[:, ta:tb, k, :],
                              op=AluOpType.add)
            if not last:
                eng.tensor_tensor(out=fb[0], in0=fb[1], in1=q, op=AluOpType.mult)

        for k in range(M - 1, -1, -1):
            for ci in range(len(spans)):
                step(ci, k, last=(k == 0))

        for c in range(4):
            dma_eng = nc.sync if c % 2 == 0 else nc.scalar
            dma_eng.dma_start(
                out=OUT.rearrange("(t p) d -> p t d", p=128)[:, 4 * c:4 * c + 4, :],
                in_=out_sb[:, 4 * c:4 * c + 4, :])

    nc.finalize()
    _built["nc"] = nc
    return nc


def _host_prep(x, Wq, bq, Wk, bk, Wv, bv):
    """Fold positional encoding + scale into weights; build constants."""
    x = np.ascontiguousarray(x, dtype=np.float32)
    Wq = np.asarray(Wq, np.float32); bq = np.asarray(bq, np.float32)
    Wk = np.asarray(Wk, np.float32); bk = np.asarray(bk, np.float32)
    Wv = np.asarray(Wv, np.float32); bv = np.asarray(bv, np.float32)

    half = D // 2
    div = np.exp(np.arange(half, dtype=np.float64) * (-np.log(10000.0) / D))
    pe = np.zeros(D, np.float64)
    pe[0::2] = np.sin(np.arange(0, D, 2, dtype=np.float64) * div)
    pe[1::2] = np.cos(np.arange(1, D, 2, dtype=np.float64) * div)
    pe = pe.astype(np.float32)

    s = np.float32(1.0 / np.sqrt(D))
    Wq_s = (Wq * s).astype(np.float32)
    bq_s = (s * (bq + Wq @ pe)).astype(np.float32)
    bk_s = (bk + Wk @ pe).astype(np.float32)
    bv_s = (bv + Wv @ pe).astype(np.float32)

    # q' range for the Chebyshev interval
    Qp = x @ Wq_s.T + bq_s
    Tmax = float(np.abs(Qp).max()) * 1.0005

    theta = (2 * np.arange(M) + 1) * np.pi / (2 * M)
    tm = np.cos(theta) * Tmax                        # f64 Chebyshev points
    Vand = tm[:, None] ** np.arange(M)[None, :]
    Ainv = np.linalg.inv(Vand)                       # coeffs = Ainv @ g_samples

    masks = np.zeros((D, M, 32), np.float32)
    for mm in range(M):
        masks[:, mm, mm] = 1.0            # stream m -> in-group partition m
    fmask = np.zeros((8, D, D), np.float32)
    for j in range(4):
        fmask[j, :, 32 * j] = 1.0         # num m=0 -> partition 32j; other rows 0
        fmask[4 + j, :, :] = 1.0          # den m=0 -> every row gets a positive sum
        fmask[4 + j, :, 32 * j + 1:32 * j + M] = 0.0   # rows for m>=1 accumulate cleanly
    tms = np.tile(tm.astype(np.float32)[None, :], (D, 1))

    # rsel[32j+m, (k,i)] = Ainv[k, m]  (zero for m >= M): ckf = g_rows.T @ rsel
    rsel = np.zeros((D, M, 128), np.float32)
    for j in range(4):
        for m in range(M):
            for k in range(M):
                rsel[32 * j + m, k, :] = Ainv[k, m]

    blob32 = np.concatenate([
        np.eye(D, dtype=np.float32),                                # IDN
        tms,                                                        # TMS
        np.stack([bk_s, bv_s], axis=1),                             # BIASCOL
    ], axis=1).astype(np.float32)

    import ml_dtypes
    blobb = np.concatenate([
        np.ascontiguousarray(Wq_s.T),                               # WQT
        np.ascontiguousarray(Wk.T),                                 # WKT
        np.ascontiguousarray(Wv.T),                                 # WVT
        fmask.transpose(1, 0, 2).reshape(D, 8 * D),                 # FMASK
        masks.reshape(D, M * 32),                                   # MASKS
        np.tile(bq_s[None, :], (D, 1)),                             # BQB
        rsel.reshape(D, M * 128),                                   # RSEL
    ], axis=1).astype(ml_dtypes.bfloat16)

    consts = {"CONSTS": np.ascontiguousarray(blob32),
              "CONSTB": np.ascontiguousarray(blobb)}
    return x, consts


def _run(inputs, trace=False):
    from concourse.bass_utils import run_bass_kernel_spmd
    x, consts = _host_prep(**inputs)
    nc = _build()
    in_maps = []
    for i in range(NCORES):
        m = {"xs": np.ascontiguousarray(x[i * NLOC:(i + 1) * NLOC])}
        m.update(consts)
        in_maps.append(m)
    res = run_bass_kernel_spmd(nc, in_maps, list(range(NCORES)), trace=trace)
    out = np.concatenate([r["out"] for r in res.results], axis=0)
    return out, res.exec_time_ns


def kernel(**inputs):
    out, _ = _run(inputs, trace=False)
    return out


# revision 19
# speedup vs baseline: 1.0878x; 1.0226x over previous
"""Trainium2 Bass kernel for per-node rank-1 self-attention (NodeFeatureSelfAttention).

Math: for each node n (row of x):
    q = s*(Wq @ xp + bq); k = Wk @ xp + bk; v = Wv @ xp + bv   (xp = x + pe)
    out[i] = sum_j softmax_j(q_i * k_j)[j] * v_j = g(q_i)
with g(t) = sum_j exp(t*k_j)*v_j / sum_j exp(t*k_j), a smooth scalar function
per node. We sample g at M shared Chebyshev points t_m (ACT engine exps),
reduce with masked-ones matmuls on the PE, turn samples directly into
broadcast monomial-coefficient planes ckf[node, tile, k, i] with one small
PE matmul per tile (Ainv folded into a constant selection matrix), and
evaluate the degree-(M-1) interpolant with full-width tensor_tensor Horner
steps column-split across DVE (bf16) and GPSIMD.

Data-parallel over nodes across 8 NeuronCores; weights replicated.
"""
import sys
sys.path.insert(0, "/opt/trn_rl_repo")
import numpy as np
from contextlib import ExitStack

N, D = 16384, 128
NCORES = 8
NLOC = N // NCORES            # 2048 nodes per core
NT = NLOC // 128              # 16 node-tiles per core
M = 4                         # Chebyshev sample count (degree M-1 interpolant)
NT_GP = 4                     # leading tiles evaluated on GPSIMD
NT_DA = 6                     # next tiles on DVE chain A (rest on chain B)

_built = {}


def _build():
    """Build + finalize the (data-independent) bass module once."""
    if "nc" in _built:
        return _built["nc"]
    import concourse.bacc as bacc
    import concourse.tile as tile
    from concourse import mybir

    f32 = mybir.dt.float32
    bf16 = mybir.dt.bfloat16
    nc = bacc.Bacc()

    xs = nc.declare_dram_parameter("xs", [NLOC, D], f32, isOutput=False)
    # f32 consts: IDN D | TMS M | BIASCOL 2
    NC32 = D + M + 2
    CONSTS = nc.declare_dram_parameter("CONSTS", [D, NC32], f32, isOutput=False)
    # bf16 consts: WQT D | WKT D | WVT D | FMASK 8D | MASKS 32M | BQB D | RSEL M*128
    NCB = 3 * D + 8 * D + 32 * M + D + M * 128
    CONSTB = nc.declare_dram_parameter("CONSTB", [D, NCB], bf16, isOutput=False)
    OUT = nc.declare_dram_parameter("out", [NLOC, D], f32, isOutput=True)

    with tile.TileContext(nc) as tc, ExitStack() as ctx:
        from concourse.mybir import AluOpType
        singles = ctx.enter_context(tc.tile_pool(name="singles", bufs=1))
        emp = ctx.enter_context(tc.tile_pool(name="emp", bufs=4))
        evp = ctx.enter_context(tc.tile_pool(name="evp", bufs=4))

        # ---- DMA in: x chunk 0 first (transposes), consts split across queues
        x_sb = singles.tile([D, NT, D], f32)
        xs_r = xs.rearrange("(t p) d -> p t d", p=128)
        cblob = singles.tile([D, NC32], f32)
        bblob = singles.tile([D, NCB], bf16)
        nc.sync.dma_start(out=x_sb[:, 0:4, :], in_=xs_r[:, 0:4, :])
        nc.scalar.dma_start(out=cblob[:, :], in_=CONSTS[:, :])
        nc.sync.dma_start(out=x_sb[:, 4:8, :], in_=xs_r[:, 4:8, :])
        nc.scalar.dma_start(out=bblob[:, 0:12 * D], in_=CONSTB[:, 0:12 * D])
        nc.sync.dma_start(out=x_sb[:, 8:12, :], in_=xs_r[:, 8:12, :])
        nc.scalar.dma_start(out=bblob[:, 12 * D:NCB], in_=CONSTB[:, 12 * D:NCB])
        nc.sync.dma_start(out=x_sb[:, 12:16, :], in_=xs_r[:, 12:16, :])

        o = 0
        idn = cblob[:, o:o + D]; o += D
        tms = cblob[:, o:o + M]; o += M
        biascol = cblob[:, o:o + 2]; o += 2
        ob = 0
        wqt = bblob[:, ob:ob + D]; ob += D
        wkt = bblob[:, ob:ob + D]; ob += D
        wvt = bblob[:, ob:ob + D]; ob += D
        fmask = bblob[:, ob:ob + 8 * D].rearrange("p (i c) -> p i c", i=8); ob += 8 * D
        masks = bblob[:, ob:ob + 32 * M].rearrange("p (i c) -> p i c", i=M); ob += 32 * M
        bqb = bblob[:, ob:ob + D]; ob += D
        rsel = bblob[:, ob:ob + M * 128]   # [32grp+m, (k,i)] = Ainv[k,m], per grp
        ob += M * 128

        xT_all = singles.tile([D, NT, 128], bf16)     # x^T per tile (bf16)
        q_all = singles.tile([D, NT, 128], bf16)      # Q' [node_p, t, i]
        kvt = singles.tile([D, NLOC], bf16)           # K^T [j, n]
        vt_b = singles.tile([D, NLOC], bf16)          # V^T
        rden = singles.tile([D, NLOC], f32)
        g_sb = singles.tile([D, NLOC], bf16)
        ckf = singles.tile([D, NT, M, 128], bf16)     # coeff planes [node_p, t, k, i]
        out_sb = singles.tile([D, NT, 128], f32)

        psA_cm = tc.tile_pool(name="psA", bufs=1, space="PSUM")
        psA = psA_cm.__enter__()

        # ---- PE warm-up: ~3.8us of dummy matmuls during the input-DMA wait
        # keeps the HAM clock gate at 8/8 (2.4 GHz) for the real matmuls.
        warm_sb = singles.tile([D, 512], bf16)
        nc.vector.memset(warm_sb[:, :], 0.0)
        for i in range(9):
            w_ps = psA.tile([128, 512], f32, tag="kps", name=f"warm{i}", bufs=2)
            nc.tensor.matmul(w_ps, warm_sb[:, 0:128], warm_sb, start=True, stop=True)

        # ---- Phase A1: transposes + K/V (kvt complete ASAP for phase B) ----
        for qd in range(4):
            for t in range(4 * qd, 4 * qd + 4):
                xt_ps = psA.tile([D, 128], f32, tag=f"xtps{t % 2}", name=f"xtps{t}")
                nc.tensor.transpose(xt_ps, x_sb[:, t, :], idn)
                nc.vector.tensor_copy(xT_all[:, t, :], xt_ps)
            xT4 = xT_all[:, 4 * qd:4 * qd + 4, :]
            nsl = slice(qd * 512, (qd + 1) * 512)
            k_ps = psA.tile([128, 512], f32, tag="kps", name=f"kps{qd}", bufs=2)
            v_ps = psA.tile([128, 512], f32, tag="vps", name=f"vps{qd}", bufs=2)
            nc.tensor.matmul(k_ps, wkt, xT4, start=True, stop=True)
            nc.tensor.matmul(v_ps, wvt, xT4, start=True, stop=True)
            nc.scalar.add(out=kvt[:, nsl], in_=k_ps, add=biascol[:, 0:1])
            nc.scalar.add(out=vt_b[:, nsl], in_=v_ps, add=biascol[:, 1:2])

        # ---- Phase A2: Q' tiles (needed only in phase D); bias via DVE add ----
        for t in range(NT):
            q_ps = psA.tile([128, D], f32, tag=f"qps{t % 2}", name=f"qps{t}")
            nc.tensor.matmul(q_ps, xT_all[:, t, :], wqt, start=True, stop=True)
            nc.vector.tensor_tensor(out=q_all[:, t, :], in0=q_ps, in1=bqb,
                                    op=AluOpType.add)
        psA_cm.__exit__(None, None, None)

        # ---- Phase B: half-split m-major exps + masked reduction matmuls ----
        psB_cm = tc.tile_pool(name="psB", bufs=1, space="PSUM")
        psB = psB_cm.__enter__()
        coef_ps = psB.tile([D, 2, NLOC], f32)
        for h in range(2):
            hsl = slice(h * 1024, (h + 1) * 1024)
            for m in range(M):
                em = emp.tile([D, 1024], bf16, name=f"em{m}h{h}")
                nc.scalar.activation(out=em, in_=kvt[:, hsl],
                                     func=mybir.ActivationFunctionType.Exp,
                                     scale=tms[:, m:m + 1])
                ev = evp.tile([D, 1024], bf16, name=f"ev{m}h{h}")
                nc.vector.tensor_mul(ev, em, vt_b[:, hsl])
                for jj in range(2):
                    j = 2 * h + jj
                    sl = slice(j * 512, (j + 1) * 512)
                    lsl = slice(jj * 512, (jj + 1) * 512)
                    if m == 0:
                        nc.tensor.matmul(coef_ps[:, 0, sl], fmask[:, j, :], ev[:, lsl],
                                         start=True, stop=False)
                    else:
                        nc.tensor.matmul(coef_ps[32 * j:32 * j + 32, 0, sl],
                                         masks[:, m, :], ev[:, lsl],
                                         start=False, stop=(m == M - 1),
                                         tile_position=(0, 32 * j))
                for jj in range(2):
                    j = 2 * h + jj
                    sl = slice(j * 512, (j + 1) * 512)
                    lsl = slice(jj * 512, (jj + 1) * 512)
                    if m == 0:
                        nc.tensor.matmul(coef_ps[:, 1, sl], fmask[:, 4 + j, :], em[:, lsl],
                                         start=True, stop=False)
                    else:
                        nc.tensor.matmul(coef_ps[32 * j:32 * j + 32, 1, sl],
                                         masks[:, m, :], em[:, lsl],
                                         start=False, stop=(m == M - 1),
                                         tile_position=(0, 32 * j))

        # ---- Phase C: g = num/den from PSUM, then per-tile ckf planes ----
        for j in range(4):
            nsl = slice(j * 512, (j + 1) * 512)
            nc.vector.reciprocal_approx_fast(out=rden[:, nsl], in_=coef_ps[:, 1, nsl])
            nc.vector.tensor_mul(g_sb[:, nsl], coef_ps[:, 0, nsl], rden[:, nsl])
        psB_cm.__exit__(None, None, None)
        psD = ctx.enter_context(tc.tile_pool(name="psD", bufs=1, space="PSUM"))
        for t in range(NT):
            j = t // 4
            ckf_ps = psD.tile([128, M * 128], f32, tag=f"ckfps{t % 4}", name=f"ckfps{t}")
            kw = {}
            if j == 3:
                kw["tile_position"] = (96, 0)
            nc.tensor.matmul(ckf_ps, g_sb[32 * j:32 * j + 32, t * 128:(t + 1) * 128],
                             rsel[32 * j:32 * j + 32, :], start=True, stop=True, **kw)
            if t % 2 == 0:
                nc.scalar.copy(out=ckf[:, t, :, :], in_=ckf_ps)
            else:
                nc.vector.tensor_copy(ckf[:, t, :, :], ckf_ps)

        # ---- Phase D: full-width Horner, column-split GPSIMD / DVE x2 ----
        hor = ctx.enter_context(tc.tile_pool(name="hor", bufs=1))
        spans = [(nc.gpsimd, 0, NT_GP),
                 (nc.vector, NT_GP, NT_GP + NT_DA),
                 (nc.vector, NT_GP + NT_DA, NT)]
        chains = []
        for ci, (eng, ta, tb) in enumerate(spans):
            nt = tb - ta
            f0 = hor.tile([D, nt, 128], bf16, name=f"f{ci}0")
            f1 = hor.tile([D, nt, 128], bf16, name=f"f{ci}1")
            chains.append((eng, ta, tb, [f0, f1]))

        def step(ci, k, last):
            eng, ta, tb, fb = chains[ci]
            q = q_all[:, ta:tb, :]
            if k == M - 1:
                eng.tensor_tensor(out=fb[0], in0=q, in1=ckf[:, ta:tb, k, :],
                                  op=AluOpType.mult)
                return
            dst = out_sb[:, ta:tb, :] if last else fb[1]
            eng.tensor_tensor(out=dst, in0=fb[0], in1=ckf[:, ta:tb, k, :],
                              op=AluOpType.add)
            if not last:
                eng.tensor_tensor(out=fb[0], in0=fb[1], in1=q, op=AluOpType.mult)

        for k in range(M - 1, -1, -1):
            for ci in range(len(spans)):
                step(ci, k, last=(k == 0))

        for c in range(4):
            dma_eng = nc.sync if c % 2 == 0 else nc.scalar
            dma_eng.dma_start(
                out=OUT.rearrange("(t p) d -> p t d", p=128)[:, 4 * c:4 * c + 4, :],
                in_=out_sb[:, 4 * c:4 * c + 4, :])

    nc.finalize()
    _built["nc"] = nc
    return nc


def _host_prep(x, Wq, bq, Wk, bk, Wv, bv):
    """Fold positional encoding + scale into weights; build constants."""
    x = np.ascontiguousarray(x, dtype=np.float32)
    Wq = np.asarray(Wq, np.float32); bq = np.asarray(bq, np.float32)
    Wk = np.asarray(Wk, np.float32); bk = np.asarray(bk, np.float32)
    Wv = np.asarray(Wv, np.float32); bv = np.asarray(bv, np.float32)

    half = D // 2
    div = np.exp(np.arange(half, dtype=np.float64) * (-np.log(10000.0) / D))
    pe = np.zeros(D, np.float64)
    pe[0::2] = np.sin(np.arange(0, D, 2, dtype=np.float64) * div)
    pe[1::2] = np.cos(np.arange(1, D, 2, dtype=np.float64) * div)
    pe = pe.astype(np.float32)

    s = np.float32(1.0 / np.sqrt(D))
    Wq_s = (Wq * s).astype(np.float32)
    bq_s = (s * (bq + Wq @ pe)).astype(np.float32)
    bk_s = (bk + Wk @ pe).astype(np.float32)
    bv_s = (bv + Wv @ pe).astype(np.float32)

    # q' range for the Chebyshev interval
    Qp = x @ Wq_s.T + bq_s
    Tmax = float(np.abs(Qp).max()) * 1.0005

    theta = (2 * np.arange(M) + 1) * np.pi / (2 * M)
    tm = np.cos(theta) * Tmax                        # f64 Chebyshev points
    Vand = tm[:, None] ** np.arange(M)[None, :]
    Ainv = np.linalg.inv(Vand)                       # coeffs = Ainv @ g_samples

    masks = np.zeros((D, M, 32), np.float32)
    for mm in range(M):
        masks[:, mm, mm] = 1.0            # stream m -> in-group partition m
    fmask = np.zeros((8, D, D), np.float32)
    for j in range(4):
        fmask[j, :, 32 * j] = 1.0         # num m=0 -> partition 32j; other rows 0
        fmask[4 + j, :, :] = 1.0          # den m=0 -> every row gets a positive sum
        fmask[4 + j, :, 32 * j + 1:32 * j + M] = 0.0   # rows for m>=1 accumulate cleanly
    tms = np.tile(tm.astype(np.float32)[None, :], (D, 1))

    # rsel[32j+m, (k,i)] = Ainv[k, m]  (zero for m >= M): ckf = g_rows.T @ rsel
    rsel = np.zeros((D, M, 128), np.float32)
    for j in range(4):
        for m in range(M):
            for k in range(M):
                rsel[32 * j + m, k, :] = Ainv[k, m]

    blob32 = np.concatenate([
        np.eye(D, dtype=np.float32),                                # IDN
        tms,                                                        # TMS
        np.stack([bk_s, bv_s], axis=1),                             # BIASCOL
    ], axis=1).astype(np.float32)

    import ml_dtypes
    blobb = np.concatenate([
        np.ascontiguousarray(Wq_s.T),                               # WQT
        np.ascontiguousarray(Wk.T),                                 # WKT
        np.ascontiguousarray(Wv.T),                                 # WVT
        fmask.transpose(1, 0, 2).reshape(D, 8 * D),                 # FMASK
        masks.reshape(D, M * 32),                                   # MASKS
        np.tile(bq_s[None, :], (D, 1)),                             # BQB
        rsel.reshape(D, M * 128),                                   # RSEL
    ], axis=1).astype(ml_dtypes.bfloat16)

    consts = {"CONSTS": np.ascontiguousarray(blob32),
              "CONSTB": np.ascontiguousarray(blobb)}
    return x, consts


def _run(inputs, trace=False):
    from concourse.bass_utils import run_bass_kernel_spmd
    x, consts = _host_prep(**inputs)
    nc = _build()
    in_maps = []
    for i in range(NCORES):
        m = {"xs": np.ascontiguousarray(x[i * NLOC:(i + 1) * NLOC])}
        m.update(consts)
        in_maps.append(m)
    res = run_bass_kernel_spmd(nc, in_maps, list(range(NCORES)), trace=trace)
    out = np.concatenate([r["out"] for r in res.results], axis=0)
    return out, res.exec_time_ns


def kernel(**inputs):
    out, _ = _run(inputs, trace=False)
    return out
